# revision 2
# baseline (speedup 1.0000x reference)
"""Coordinate-descent (alternating Gauss-Seidel) kernel for Trainium2.

Problem: B=4 independent factorizations x ~ u @ v^T with M=N=4096, R=32.
  u_new = GS-sweep(a1 = x@v,   b1 = v^T v, u)
  v_new = GS-sweep(a2 = x^T@u_new, b2 = u_new^T u_new, v)

Sharding: 8 cores, each owns rows [c*512,(c+1)*512) of ALL 4 batches for u,
and the same index range of n-rows for v (delivered by a single
ReduceScatter of the concatenated per-batch partial a2/b2 products).

Stages (all batches each):
  phase1: stream x fp32 tiles -> cast bf16 (kept in SBUF for phase 2),
          PE-transpose -> x^T tiles, a1T = sum_c v_c^T @ xT_c  (bf16 MMs)
  u-GS:   batched (all 4 batches) s-incremental Gauss-Seidel sweep on DVE
  phase2: a2_partial natural = sum_i x_nat_i^T @ u_new_i (bf16 MMs),
          b2_partial; ONE ReduceScatter over 8 cores for all batches
  v-GS:   batched sweep on the scattered slices
"""

import os
from contextlib import ExitStack

import numpy as np

import concourse.bass as bass
import concourse.tile as tile
from concourse import bacc, mybir
from concourse.bass import ds
from concourse.bass_utils import run_bass_kernel_spmd
from concourse.masks import make_identity

B, M, N, R = 4, 4096, 4096, 32
NCORES = 8
MS = M // NCORES          # 512 rows per core per batch
MC = MS // 128            # 4 m-chunks of 128
NG = N // 512             # 8 n-groups of 512
NCH = N // 128            # 32 n-chunks of 128
BMC = B * MC              # 16
CHUNK = B * (MS + R)      # 2176 rows per core in the fused ReduceScatter
EPS = 1e-8
FP32 = mybir.dt.float32
BF16 = mybir.dt.bfloat16
ALU = mybir.AluOpType

_CACHE = {}
LAST_RESULT = None


def _gs_sweep_batched(nc, gsp, pmisc, ident_f, u_all, a_all, b_sbs, ball,
                      unew):
    """Batched Gauss-Seidel sweep over all B batches at once.

    u_all: [128, BMC, R] fp32 current factor rows (free = b*MC+i)
    a_all: [128, BMC, R] fp32
    b_sbs: list of B [R, R] fp32 gram matrices (natural, SBUF)
    ball:  [128, B, R, R] fp32 grams replicated on every partition
    unew:  [128, BMC, R] fp32 output AP
    """
    # s = u @ b per batch (via per-batch u^T PE transposes + K=32 matmuls)
    s = gsp.tile([128, BMC, R], FP32, tag="s")
    for b in range(B):
        puT = pmisc.tile([R, MC, 128], FP32, tag="pm")
        for i in range(MC):
            nc.tensor.transpose(puT[:, i], u_all[:, b * MC + i, :], ident_f)
        uT = gsp.tile([R, MC, 128], FP32, tag="uT")
        nc.vector.tensor_copy(uT[:], puT[:])
        ps = pmisc.tile([128, MC, R], FP32, tag="pm")
        for i in range(MC):
            nc.tensor.matmul(ps[:, i], lhsT=uT[:, i], rhs=b_sbs[b][:],
                             start=True, stop=True)
        nc.vector.tensor_copy(s[:, b * MC:(b + 1) * MC, :], ps[:])

    # brr16/inv16: per-(batch, r) diagonal replicated over the MC dim
    brr16 = gsp.tile([128, BMC, R], FP32, tag="brr16")
    bap = ball[:]
    for b in range(B):
        diag_bc = bass.AP(bap.tensor, bap.offset + b * R * R,
                          [bap.ap[0], [0, MC], [R + 1, R]])
        nc.vector.tensor_copy(brr16[:, b * MC:(b + 1) * MC, :], diag_bc)
    inv16 = gsp.tile([128, BMC, R], FP32, tag="inv16")
    nc.vector.tensor_scalar_add(inv16[:], brr16[:], EPS)
    nc.vector.reciprocal(inv16[:], inv16[:])

    app = gsp.tile([128, BMC, R], FP32, tag="app")
    nc.vector.scalar_tensor_tensor(out=app[:], in0=a_all[:], scalar=EPS,
                                   in1=inv16[:], op0=ALU.add, op1=ALU.mult)

    t1 = gsp.tile([128, BMC], FP32, tag="t1")
    delta = gsp.tile([128, BMC, 1], FP32, tag="delta")
    tmp = gsp.tile([128, BMC, R - 1], FP32, tag="tmp")
    dap = delta[:]
    tap = tmp[:]
    for r in range(R):
        nc.vector.tensor_tensor(out=t1[:], in0=u_all[:, :, r],
                                in1=brr16[:, :, r], op=ALU.mult)
        nc.vector.tensor_tensor(out=t1[:], in0=t1[:], in1=s[:, :, r],
                                op=ALU.subtract)
        nc.vector.tensor_tensor(out=t1[:], in0=t1[:], in1=inv16[:, :, r],
                                op=ALU.mult)
        nc.vector.tensor_tensor(out=unew[:, :, r], in0=t1[:],
                                in1=app[:, :, r], op=ALU.add)
        if r < R - 1:
            tail = R - 1 - r
            nc.vector.tensor_tensor(out=delta[:, :, 0], in0=unew[:, :, r],
                                    in1=u_all[:, :, r], op=ALU.subtract)
            # tmp[b, i, t] = delta[b, i] * ball[b, r, r+1+t]
            d_bc = bass.AP(dap.tensor, dap.offset,
                           [dap.ap[0], [MC, B], [1, MC], [0, tail]])
            brow_bc = bass.AP(bap.tensor, bap.offset + r * R + r + 1,
                              [bap.ap[0], [R * R, B], [0, MC], [1, tail]])
            t_out = bass.AP(tap.tensor, tap.offset,
                            [tap.ap[0], [MC * (R - 1), B], [R - 1, MC],
                             [1, tail]])
            nc.vector.tensor_tensor(out=t_out, in0=d_bc, in1=brow_bc,
                                    op=ALU.mult)
            nc.vector.tensor_tensor(out=s[:, :, r + 1:], in0=s[:, :, r + 1:],
                                    in1=tmp[:, :, :tail], op=ALU.add)


def _build():
    nc = bacc.Bacc("TRN2", target_bir_lowering=False, debug=False,
                   num_devices=NCORES)

    x_my = nc.dram_tensor("x_my", [B, MS, N], FP32, kind="ExternalInput").ap()
    u_my = nc.dram_tensor("u_my", [B, MS, R], FP32, kind="ExternalInput").ap()
    v_full = nc.dram_tensor("v_full", [B, N, R], FP32,
                            kind="ExternalInput").ap()
    v_my = nc.dram_tensor("v_my", [B, MS, R], FP32, kind="ExternalInput").ap()
    u_out = nc.dram_tensor("u_out", [B, MS, R], FP32,
                           kind="ExternalOutput").ap()
    v_out = nc.dram_tensor("v_out", [B, MS, R], FP32,
                           kind="ExternalOutput").ap()

    rs_in = nc.dram_tensor("rs_in", [NCORES * CHUNK, R], FP32)
    rs_out = nc.dram_tensor("rs_out", [CHUNK, R], FP32)
    b1_scr = nc.dram_tensor("b1_scr", [B, R, R], FP32)

    with tile.TileContext(nc) as tc, ExitStack() as ctx:
        const = ctx.enter_context(tc.tile_pool(name="const", bufs=1))
        big = ctx.enter_context(tc.tile_pool(name="big", bufs=1))
        xl = ctx.enter_context(tc.tile_pool(name="xl", bufs=3))
        xt = ctx.enter_context(tc.tile_pool(name="xt", bufs=2))
        vpool = ctx.enter_context(tc.tile_pool(name="vp", bufs=1))
        gsp = ctx.enter_context(tc.tile_pool(name="gsp", bufs=1))
        sm = ctx.enter_context(tc.tile_pool(name="sm", bufs=2))
        ppt = ctx.enter_context(tc.tile_pool(name="ppt", bufs=2, space="PSUM"))
        pa1p = ctx.enter_context(tc.tile_pool(name="pa1", bufs=2,
                                              space="PSUM"))
        pa2p = ctx.enter_context(tc.tile_pool(name="pa2", bufs=2,
                                              space="PSUM"))
        pmisc = ctx.enter_context(tc.tile_pool(name="pmisc", bufs=2,
                                               space="PSUM"))

        ident_b = const.tile([128, 128], BF16)
        make_identity(nc, ident_b)
        ident_f = const.tile([128, 128], FP32)
        make_identity(nc, ident_f)

        x_nat = big.tile([128, BMC, N], BF16)         # 16.8 MB persistent
        unew_all = big.tile([128, BMC, R], FP32)
        ball = big.tile([128, B, R, R], FP32)         # grams, replicated
        u_all = big.tile([128, BMC, R], FP32)
        a_all = big.tile([128, BMC, R], FP32)

        b1_sbs = []
        for b in range(B):
            # ---------- v load + b1 = v^T v ----------
            v32 = vpool.tile([128, NCH, R], FP32, tag="v32")
            nc.sync.dma_start(v32[:],
                              v_full[b].rearrange("(c p) r -> p c r", p=128))
            vb = vpool.tile([128, NCH, R], BF16, tag="vb")
            nc.vector.tensor_copy(vb[:], v32[:])

            pb1 = pmisc.tile([R, R], FP32, tag="pm")
            for c in range(NCH):
                nc.tensor.matmul(pb1[:], lhsT=vb[:, c], rhs=vb[:, c],
                                 start=(c == 0), stop=(c == NCH - 1))
            b1_sb = sm.tile([R, R], FP32, tag=f"b1_{b}")
            nc.vector.tensor_copy(b1_sb[:], pb1[:])
            b1_sbs.append(b1_sb)
            nc.sync.dma_start(b1_scr.ap()[b], b1_sb[:])
            src = b1_scr.ap()[b]
            nc.sync.dma_start(
                ball[:, b], bass.AP(src.tensor, src.offset,
                                    [[0, 128], [R, R], [1, R]]))

            # ---------- phase 1: stream x, transpose, a1T ----------
            pa1 = pa1p.tile([R, MS], FP32, tag="pa1")
            for j in range(NG):
                xT = xt.tile([128, 4, MS], BF16, tag="xT")
                for i in range(MC):
                    xload = xl.tile([128, 512], FP32, tag="xload")
                    nc.sync.dma_start(
                        xload[:],
                        x_my[b, i * 128:(i + 1) * 128, j * 512:(j + 1) * 512])
                    nc.scalar.copy(
                        x_nat[:, b * MC + i, j * 512:(j + 1) * 512], xload[:])
                    pt = ppt.tile([128, 4, 128], BF16, tag="pt")
                    for k in range(4):
                        nc.tensor.transpose(
                            pt[:, k],
                            x_nat[:, b * MC + i,
                                  (j * 4 + k) * 128:(j * 4 + k + 1) * 128],
                            ident_b)
                    nc.scalar.copy(xT[:, :, i * 128:(i + 1) * 128], pt[:])
                for k in range(4):
                    c = j * 4 + k
                    nc.tensor.matmul(pa1[:], lhsT=vb[:, c], rhs=xT[:, k],
                                     start=(c == 0), stop=(c == NCH - 1))
            a1T_sb = sm.tile([R, MS], FP32, tag="a1T")
            nc.vector.tensor_copy(a1T_sb[:], pa1[:])

            # a natural + u load into the batched tiles
            nc.sync.dma_start(u_all[:, b * MC:(b + 1) * MC, :],
                              u_my[b].rearrange("(i p) r -> p i r", p=128))
            pA = pmisc.tile([128, MC, R], FP32, tag="pm")
            for i in range(MC):
                nc.tensor.transpose(pA[:, i],
                                    a1T_sb[:, i * 128:(i + 1) * 128],
                                    ident_f[:R, :R])
            nc.vector.tensor_copy(a_all[:, b * MC:(b + 1) * MC, :], pA[:])

        # ---------- batched u GS ----------
        _gs_sweep_batched(nc, gsp, pmisc, ident_f, u_all, a_all, b1_sbs,
                          ball, unew_all[:])
        for b in range(B):
            nc.sync.dma_start(u_out[b].rearrange("(i p) r -> p i r", p=128),
                              unew_all[:, b * MC:(b + 1) * MC, :])
        un_b = sm.tile([128, BMC, R], BF16, tag="unb")
        nc.vector.tensor_copy(un_b[:], unew_all[:])

        # ---------- phase 2 + fused ReduceScatter ----------
        for b in range(B):
            for g in range(NG):
                pa2 = pa2p.tile([128, 4, R], FP32, tag="pa2")
                for k in range(4):
                    nblk = g * 4 + k
                    for i in range(MC):
                        nc.tensor.matmul(
                            pa2[:, k],
                            lhsT=x_nat[:, b * MC + i,
                                       nblk * 128:(nblk + 1) * 128],
                            rhs=un_b[:, b * MC + i], start=(i == 0),
                            stop=(i == MC - 1))
                a2st = sm.tile([128, 4, R], FP32, tag="a2st")
                nc.vector.tensor_copy(a2st[:], pa2[:])
                dst = rs_in.ap()
                nc.sync.dma_start(
                    bass.AP(dst.tensor,
                            dst.offset + (g * CHUNK + b * (MS + R)) * R,
                            [[R, 128], [128 * R, 4], [1, R]]),
                    a2st[:])

            pb2 = pmisc.tile([R, R], FP32, tag="pm")
            for i in range(MC):
                nc.tensor.matmul(pb2[:], lhsT=un_b[:, b * MC + i],
                                 rhs=un_b[:, b * MC + i], start=(i == 0),
                                 stop=(i == MC - 1))
            b2st = sm.tile([R, R], FP32, tag="b2st")
            nc.vector.tensor_copy(b2st[:], pb2[:])
            for c in range(NCORES):
                nc.sync.dma_start(
                    rs_in.ap()[ds(c * CHUNK + b * (MS + R) + MS, R), :],
                    b2st[:])

        nc.gpsimd.collective_compute(
            "ReduceScatter", ALU.add, replica_groups=[list(range(NCORES))],
            ins=[rs_in.ap()], outs=[rs_out.ap()])

        # ---------- batched v GS ----------
        v_all = big.tile([128, BMC, R], FP32)
        a2_all = big.tile([128, BMC, R], FP32)
        b2_sbs = []
        for b in range(B):
            nc.sync.dma_start(v_all[:, b * MC:(b + 1) * MC, :],
                              v_my[b].rearrange("(i p) r -> p i r", p=128))
            nc.sync.dma_start(
                a2_all[:, b * MC:(b + 1) * MC, :],
                rs_out.ap()[ds(b * (MS + R), MS), :].rearrange(
                    "(i p) r -> p i r", p=128))
            b2_sb = sm.tile([R, R], FP32, tag=f"b2_{b}")
            nc.sync.dma_start(b2_sb[:],
                              rs_out.ap()[ds(b * (MS + R) + MS, R), :])
            b2_sbs.append(b2_sb)
            src = rs_out.ap()
            nc.sync.dma_start(
                ball[:, b],
                bass.AP(src.tensor, src.offset + (b * (MS + R) + MS) * R,
                        [[0, 128], [R, R], [1, R]]))

        vnew = big.tile([128, BMC, R], FP32)
        _gs_sweep_batched(nc, gsp, pmisc, ident_f, v_all, a2_all, b2_sbs,
                          ball, vnew[:])
        for b in range(B):
            nc.sync.dma_start(v_out[b].rearrange("(i p) r -> p i r", p=128),
                              vnew[:, b * MC:(b + 1) * MC, :])

    nc.compile()
    return nc


def kernel(x, u, v):
    global LAST_RESULT
    if "nc" not in _CACHE:
        _CACHE["nc"] = _build()
    nc = _CACHE["nc"]

    x = np.ascontiguousarray(x, dtype=np.float32)
    u = np.ascontiguousarray(u, dtype=np.float32)
    v = np.ascontiguousarray(v, dtype=np.float32)

    in_maps = []
    for c in range(NCORES):
        sl = slice(c * MS, (c + 1) * MS)
        in_maps.append({
            "x_my": np.ascontiguousarray(x[:, sl, :]),
            "u_my": np.ascontiguousarray(u[:, sl, :]),
            "v_full": v,
            "v_my": np.ascontiguousarray(v[:, sl, :]),
        })

    res = run_bass_kernel_spmd(nc, in_maps, list(range(NCORES)),
                               trace=os.environ.get("KBENCH_TRACE") == "1")
    LAST_RESULT = res
    u_new = np.concatenate([res.results[c]["u_out"] for c in range(NCORES)],
                           axis=1)
    v_new = np.concatenate([res.results[c]["v_out"] for c in range(NCORES)],
                           axis=1)
    return (u_new, v_new)


# revision 6
# speedup vs baseline: 1.1173x; 1.1173x over previous
"""Coordinate-descent (alternating Gauss-Seidel) kernel for Trainium2.

B=4 factorizations x ~ u @ v^T, M=N=4096, R=32.
  u_new = GS-sweep(a1 = x@v,   b1 = v^T v, u)
  v_new = GS-sweep(a2 = x^T@u_new, b2 = u_new^T u_new, v)

8 cores; core c owns rows [c*512,(c+1)*512) of all batches (u rows, and the
same n-range of v delivered by one fused ReduceScatter of partial a2/b2).

Pipeline: per batch: phase1 (stream x: cast bf16 -> persistent SBUF,
PE-transpose, dense a1 MM burst) -> per-batch u GS sweep -> phase2 partials
+ RS-input DMAs (all overlap the next batch's phase1). One ReduceScatter,
v-transposes hoisted before it, batched v GS sweep after.
"""

import os
from contextlib import ExitStack

import numpy as np

import concourse.bass as bass
import concourse.tile as tile
from concourse import bacc, mybir
from concourse.bass import ds
from concourse.bass_utils import run_bass_kernel_spmd
from concourse.masks import make_identity

B, M, N, R = 4, 4096, 4096, 32
NCORES = 8
MS = M // NCORES          # 512 rows per core per batch
MC = MS // 128            # 4 m-chunks of 128
NG = N // 512             # 8 n-groups of 512
NCH = N // 128            # 32 n-chunks of 128
BMC = B * MC              # 16
CHUNK = B * (MS + R)      # 2176 rows per core in the fused ReduceScatter
EPS = 1e-8
FP32 = mybir.dt.float32
BF16 = mybir.dt.bfloat16
ALU = mybir.AluOpType

_CACHE = {}
LAST_RESULT = None


def _gs_sweep(nc, gsp, pmisc, ident_f, u_ap, a_ap, b_sbs, ball_ap, nb,
              unew, pre_uT=None):
    """Gauss-Seidel sweep over nb batches at once.

    u_ap/a_ap/unew: [128, nb*MC, R] fp32 APs; b_sbs: nb [R,R] grams (SBUF);
    ball_ap: [128, nb, R, R] fp32 replicated grams; pre_uT: optional
    precomputed list of [R, MC, 128] transposed-factor tiles.
    """
    nmc = nb * MC
    s = gsp.tile([128, BMC, R], FP32, tag="s", name="s")[:, :nmc, :]
    for bb in range(nb):
        if pre_uT is None:
            puT = pmisc.tile([R, MC, 128], FP32, tag="pm")
            for i in range(MC):
                nc.tensor.transpose(puT[:, i], u_ap[:, bb * MC + i, :],
                                    ident_f)
            uT = gsp.tile([R, MC, 128], FP32, tag="uT")
            nc.vector.tensor_copy(uT[:], puT[:])
        else:
            uT = pre_uT[bb]
        ps = pmisc.tile([128, MC, R], FP32, tag="pm")
        for i in range(MC):
            nc.tensor.matmul(ps[:, i], lhsT=uT[:, i], rhs=b_sbs[bb][:],
                             start=True, stop=True)
        nc.vector.tensor_copy(s[:, bb * MC:(bb + 1) * MC, :], ps[:])

    brr16 = gsp.tile([128, BMC, R], FP32, tag="brr16", name="brr16")[:, :nmc, :]
    for bb in range(nb):
        diag_bc = bass.AP(ball_ap.tensor, ball_ap.offset + bb * R * R,
                          [ball_ap.ap[0], [0, MC], [R + 1, R]])
        nc.vector.tensor_copy(brr16[:, bb * MC:(bb + 1) * MC, :], diag_bc)
    inv16 = gsp.tile([128, BMC, R], FP32, tag="inv16", name="inv16")[:, :nmc, :]
    nc.vector.tensor_scalar_add(inv16[:], brr16[:], EPS)
    nc.vector.reciprocal(inv16[:], inv16[:])

    app = gsp.tile([128, BMC, R], FP32, tag="app", name="app")[:, :nmc, :]
    nc.vector.scalar_tensor_tensor(out=app[:], in0=a_ap, scalar=EPS,
                                   in1=inv16[:], op0=ALU.add, op1=ALU.mult)

    t1 = gsp.tile([128, BMC], FP32, tag="t1", name="t1")[:, :nmc]
    delta = gsp.tile([128, BMC, 1], FP32, tag="delta")
    tmp = gsp.tile([128, BMC, R - 1], FP32, tag="tmp")
    dap = delta[:]
    tap = tmp[:]
    for r in range(R):
        nc.vector.tensor_tensor(out=t1[:], in0=u_ap[:, :, r],
                                in1=brr16[:, :, r], op=ALU.mult)
        nc.vector.tensor_tensor(out=t1[:], in0=t1[:], in1=s[:, :, r],
                                op=ALU.subtract)
        nc.vector.tensor_tensor(out=t1[:], in0=t1[:], in1=inv16[:, :, r],
                                op=ALU.mult)
        nc.vector.tensor_tensor(out=unew[:, :, r], in0=t1[:],
                                in1=app[:, :, r], op=ALU.add)
        if r < R - 1:
            tail = R - 1 - r
            nc.vector.tensor_tensor(out=delta[:, :nmc, 0],
                                    in0=unew[:, :, r], in1=u_ap[:, :, r],
                                    op=ALU.subtract)
            d_bc = bass.AP(dap.tensor, dap.offset,
                           [dap.ap[0], [MC, nb], [1, MC], [0, tail]])
            brow_bc = bass.AP(ball_ap.tensor,
                              ball_ap.offset + r * R + r + 1,
                              [ball_ap.ap[0], [R * R, nb], [0, MC],
                               [1, tail]])
            t_out = bass.AP(tap.tensor, tap.offset,
                            [tap.ap[0], [MC * (R - 1), nb], [R - 1, MC],
                             [1, tail]])
            nc.vector.tensor_tensor(out=t_out, in0=d_bc, in1=brow_bc,
                                    op=ALU.mult)
            nc.vector.tensor_tensor(out=s[:, :, r + 1:],
                                    in0=s[:, :, r + 1:],
                                    in1=tmp[:, :nmc, :tail], op=ALU.add)


def _build():
    nc = bacc.Bacc("TRN2", target_bir_lowering=False, debug=False,
                   num_devices=NCORES)

    x_my = nc.dram_tensor("x_my", [B, MS, N], FP32, kind="ExternalInput").ap()
    u_my = nc.dram_tensor("u_my", [B, MS, R], FP32, kind="ExternalInput").ap()
    v_full = nc.dram_tensor("v_full", [B, N, R], FP32,
                            kind="ExternalInput").ap()
    v_my = nc.dram_tensor("v_my", [B, MS, R], FP32, kind="ExternalInput").ap()
    u_out = nc.dram_tensor("u_out", [B, MS, R], FP32,
                           kind="ExternalOutput").ap()
    v_out = nc.dram_tensor("v_out", [B, MS, R], FP32,
                           kind="ExternalOutput").ap()

    rs_in = nc.dram_tensor("rs_in", [NCORES * CHUNK, R], FP32)
    rs_out = nc.dram_tensor("rs_out", [CHUNK, R], FP32)
    b1_scr = nc.dram_tensor("b1_scr", [B, R, R], FP32)

    with tile.TileContext(nc) as tc, ExitStack() as ctx:
        const = ctx.enter_context(tc.tile_pool(name="const", bufs=1))
        big = ctx.enter_context(tc.tile_pool(name="big", bufs=1))
        xl = ctx.enter_context(tc.tile_pool(name="xl", bufs=4))
        xt = ctx.enter_context(tc.tile_pool(name="xt", bufs=1))
        xnatp = ctx.enter_context(tc.tile_pool(name="xnatp", bufs=2))
        vpool = ctx.enter_context(tc.tile_pool(name="vp", bufs=1))
        gsp = ctx.enter_context(tc.tile_pool(name="gsp", bufs=1))
        sm = ctx.enter_context(tc.tile_pool(name="sm", bufs=2))
        ppt = ctx.enter_context(tc.tile_pool(name="ppt", bufs=3, space="PSUM"))
        pa1p = ctx.enter_context(tc.tile_pool(name="pa1", bufs=1,
                                              space="PSUM"))
        pa2p = ctx.enter_context(tc.tile_pool(name="pa2", bufs=2,
                                              space="PSUM"))
        pmisc = ctx.enter_context(tc.tile_pool(name="pmisc", bufs=2,
                                               space="PSUM"))

        ident_b = const.tile([128, 128], BF16)
        make_identity(nc, ident_b)
        ident_f = const.tile([128, 128], FP32)
        make_identity(nc, ident_f)

        unew_all = big.tile([128, BMC, R], FP32)
        ball = big.tile([128, B, R, R], FP32)
        u_all = big.tile([128, BMC, R], FP32)
        a_all = big.tile([128, BMC, R], FP32)
        un_b = big.tile([128, BMC, R], BF16)

        b1_sbs = []
        xnat_tiles = {}
        for b in range(B):
            x_nat = xnatp.tile([128, MC, N], BF16, tag="xnat", name="xnat")
            xnat_tiles[b] = x_nat
            # ---------- v load + b1 = v^T v ----------
            v32 = vpool.tile([128, NCH, R], FP32, tag="v32")
            nc.sync.dma_start(v32[:],
                              v_full[b].rearrange("(c p) r -> p c r", p=128))
            vb = vpool.tile([128, NCH, R], BF16, tag="vb")
            nc.vector.tensor_copy(vb[:], v32[:])

            pb1 = pmisc.tile([R, R], FP32, tag="pm")
            for c in range(NCH):
                nc.tensor.matmul(pb1[:], lhsT=vb[:, c], rhs=vb[:, c],
                                 start=(c == 0), stop=(c == NCH - 1))
            b1_sb = sm.tile([R, R], FP32, tag=f"b1_{b}")
            nc.vector.tensor_copy(b1_sb[:], pb1[:])
            b1_sbs.append(b1_sb)
            nc.sync.dma_start(b1_scr.ap()[b], b1_sb[:])
            src = b1_scr.ap()[b]
            nc.sync.dma_start(
                ball[:, b], bass.AP(src.tensor, src.offset,
                                    [[0, 128], [R, R], [1, R]]))

            # ---------- phase 1: stream x, transpose; then dense MM burst ---
            xT = xt.tile([128, NCH, MS], BF16, tag="xT")
            for j in range(NG):
                for i in range(MC):
                    xload = xl.tile([128, 512], FP32, tag="xload")
                    nc.sync.dma_start(
                        xload[:],
                        x_my[b, i * 128:(i + 1) * 128, j * 512:(j + 1) * 512])
                    nc.scalar.copy(
                        x_nat[:, i, j * 512:(j + 1) * 512], xload[:])
                    pt = ppt.tile([128, 4, 128], BF16, tag="pt")
                    for k in range(4):
                        nc.tensor.transpose(
                            pt[:, k],
                            x_nat[:, i,
                                  (j * 4 + k) * 128:(j * 4 + k + 1) * 128],
                            ident_b)
                    nc.scalar.copy(
                        xT[:, j * 4:(j + 1) * 4, i * 128:(i + 1) * 128],
                        pt[:])
            pa1 = pa1p.tile([R, MS], FP32, tag="pa1")
            for c in range(NCH):
                nc.tensor.matmul(pa1[:], lhsT=vb[:, c], rhs=xT[:, c],
                                 start=(c == 0), stop=(c == NCH - 1))
            a1T_sb = sm.tile([R, MS], FP32, tag="a1T")
            nc.vector.tensor_copy(a1T_sb[:], pa1[:])

            # a natural + u load
            nc.sync.dma_start(u_all[:, b * MC:(b + 1) * MC, :],
                              u_my[b].rearrange("(i p) r -> p i r", p=128))
            pA = pmisc.tile([128, MC, R], FP32, tag="pm")
            for i in range(MC):
                nc.tensor.transpose(pA[:, i],
                                    a1T_sb[:, i * 128:(i + 1) * 128],
                                    ident_f[:R, :R])
            nc.vector.tensor_copy(a_all[:, b * MC:(b + 1) * MC, :], pA[:])

            # ---------- per-batch u GS sweep (overlaps next phase1) -------
            sl = slice(b * MC, (b + 1) * MC)
            _gs_sweep(nc, gsp, pmisc, ident_f, u_all[:, sl, :],
                      a_all[:, sl, :], [b1_sb], ball[:, b:b + 1],
                      1, unew_all[:, sl, :])
            nc.sync.dma_start(u_out[b].rearrange("(i p) r -> p i r", p=128),
                              unew_all[:, sl, :])
            nc.vector.tensor_copy(un_b[:, sl, :], unew_all[:, sl, :])

            # ---------- phase 2 partials + RS-input DMAs ------------------
            for g in range(NG):
                pa2 = pa2p.tile([128, 4, R], FP32, tag="pa2")
                for k in range(4):
                    nblk = g * 4 + k
                    for i in range(MC):
                        nc.tensor.matmul(
                            pa2[:, k],
                            lhsT=xnat_tiles[b][:, i,
                                       nblk * 128:(nblk + 1) * 128],
                            rhs=un_b[:, b * MC + i], start=(i == 0),
                            stop=(i == MC - 1))
                a2st = sm.tile([128, 4, R], FP32, tag="a2st")
                nc.vector.tensor_copy(a2st[:], pa2[:])
                dst = rs_in.ap()
                nc.sync.dma_start(
                    bass.AP(dst.tensor,
                            dst.offset + (g * CHUNK + b * (MS + R)) * R,
                            [[R, 128], [128 * R, 4], [1, R]]),
                    a2st[:])

            pb2 = pmisc.tile([R, R], FP32, tag="pm")
            for i in range(MC):
                nc.tensor.matmul(pb2[:], lhsT=un_b[:, b * MC + i],
                                 rhs=un_b[:, b * MC + i], start=(i == 0),
                                 stop=(i == MC - 1))
            b2st = sm.tile([R, R], FP32, tag="b2st")
            nc.vector.tensor_copy(b2st[:], pb2[:])
            for c in range(NCORES):
                nc.sync.dma_start(
                    rs_in.ap()[ds(c * CHUNK + b * (MS + R) + MS, R), :],
                    b2st[:])

        # ---------- v loads + transposes (overlap RS) ---------------------
        v_all = big.tile([128, BMC, R], FP32)
        vT_tiles = []
        for b in range(B):
            nc.sync.dma_start(v_all[:, b * MC:(b + 1) * MC, :],
                              v_my[b].rearrange("(i p) r -> p i r", p=128))
            pvT = pmisc.tile([R, MC, 128], FP32, tag="pm")
            for i in range(MC):
                nc.tensor.transpose(pvT[:, i], v_all[:, b * MC + i, :],
                                    ident_f)
            vT = sm.tile([R, MC, 128], FP32, tag=f"vT_{b}")
            nc.vector.tensor_copy(vT[:], pvT[:])
            vT_tiles.append(vT)

        nc.gpsimd.collective_compute(
            "ReduceScatter", ALU.add, replica_groups=[list(range(NCORES))],
            ins=[rs_in.ap()], outs=[rs_out.ap()])

        # ---------- batched v GS ------------------------------------------
        a2_all = big.tile([128, BMC, R], FP32)
        b2_sbs = []
        for b in range(B):
            nc.sync.dma_start(
                a2_all[:, b * MC:(b + 1) * MC, :],
                rs_out.ap()[ds(b * (MS + R), MS), :].rearrange(
                    "(i p) r -> p i r", p=128))
            b2_sb = sm.tile([R, R], FP32, tag=f"b2_{b}")
            nc.sync.dma_start(b2_sb[:],
                              rs_out.ap()[ds(b * (MS + R) + MS, R), :])
            b2_sbs.append(b2_sb)
            src = rs_out.ap()
            nc.sync.dma_start(
                ball[:, b],
                bass.AP(src.tensor, src.offset + (b * (MS + R) + MS) * R,
                        [[0, 128], [R, R], [1, R]]))

        vnew = big.tile([128, BMC, R], FP32)
        _gs_sweep(nc, gsp, pmisc, ident_f, v_all[:], a2_all[:], b2_sbs,
                  ball[:], B, vnew[:], pre_uT=vT_tiles)
        for b in range(B):
            nc.sync.dma_start(v_out[b].rearrange("(i p) r -> p i r", p=128),
                              vnew[:, b * MC:(b + 1) * MC, :])

    nc.compile()
    return nc


def kernel(x, u, v):
    global LAST_RESULT
    if "nc" not in _CACHE:
        _CACHE["nc"] = _build()
    nc = _CACHE["nc"]

    x = np.ascontiguousarray(x, dtype=np.float32)
    u = np.ascontiguousarray(u, dtype=np.float32)
    v = np.ascontiguousarray(v, dtype=np.float32)

    in_maps = []
    for c in range(NCORES):
        sl = slice(c * MS, (c + 1) * MS)
        in_maps.append({
            "x_my": np.ascontiguousarray(x[:, sl, :]),
            "u_my": np.ascontiguousarray(u[:, sl, :]),
            "v_full": v,
            "v_my": np.ascontiguousarray(v[:, sl, :]),
        })

    res = run_bass_kernel_spmd(nc, in_maps, list(range(NCORES)),
                               trace=os.environ.get("KBENCH_TRACE") == "1")
    LAST_RESULT = res
    u_new = np.concatenate([res.results[c]["u_out"] for c in range(NCORES)],
                           axis=1)
    v_new = np.concatenate([res.results[c]["v_out"] for c in range(NCORES)],
                           axis=1)
    return (u_new, v_new)
